# revision 5
# baseline (speedup 1.0000x reference)
"""Deformable self-attention kernel for Trainium2 (8 NeuronCores).

Structural reduction: the sampling offsets are ``tanh(...) * (2/128)`` with
``|tanh| < 1``, added to *integer* grid coordinates and then rounded.  Since
the perturbation magnitude is < 0.5, ``round(c + d) == c`` always, so the
gather indices are exactly ``arange(N)`` (identity), independent of the data.
Each token attends only to itself at all 7 points; the 7 scores are equal, so
softmax is uniform and the attention output equals ``v``.  The whole module
therefore computes

    out = (x @ Wv + bv) @ Wo + bo = x @ (Wv @ Wo) + (bv @ Wo + bo)

Device strategy (per sharding_hint, row-parallel over the N axis):
  - the weight fold W = Wv @ Wo and the bias fold are computed on host in
    fp32 (134 MFLOP total) and shipped once per core as bf16 [D, D];
  - each core gets 2048 tokens of x, fed pre-transposed ([D, T] layout) and
    pre-cast to bf16 - layout/dtype marshaling done while sharding;
  - the main [2048, 512] @ [512, 512] matmul runs in bf16 (1 cycle/row on
    the PE, fp32 PSUM accumulate), which is the PE roofline for this GEMM;
  - outputs are stored as bf16 and upcast on host; total HBM traffic per
    core is 2 MB (x) + 0.5 MB (W) + 2 MB (out) = 4.5 MB vs 10 MB for the
    fp32 version;
  - the PE clock needs ~3 us of sustained work to ramp to full speed, so a
    chain of dummy matmuls warms it up while the first DMAs land;
  - loads stream on the Sync hwdge ring (small W k0-slice first so the
    first token tile can start after ~0.6 MB, not after all 2.5 MB),
    stores stream on the Scalar hwdge ring so they overlap the loads;
  - PSUM->SBUF copies (with the fp32->bf16 cast) alternate DVE/ACT.
"""

import os
import sys

import numpy as np

for _p in ("/opt/trn_rl_repo", "/root/.axon_site/_ro/trn_rl_repo"):
    if os.path.isdir(_p) and _p not in sys.path:
        sys.path.append(_p)

import ml_dtypes
import concourse.bass as bass
import concourse.mybir as mybir
import concourse.tile as tile
from concourse import bacc
from concourse.bass_utils import run_bass_kernel_spmd
from concourse.tile import add_dep_helper

N_CORES = 8
N = 16384          # tokens (128 x 128 grid)
D = 512            # d_model
T = N // N_CORES   # tokens per core
P = 128            # partitions
TT = T // P        # token tiles per core (16)
KT = D // P        # contraction tiles (4)
POB = 4            # main psum bufs
NWARM = 18         # PE clock warm-up matmuls (bridge until first x lands)
WARM_F = 256       # warm-up moving size
# input DMA split: small chunks first so the first token tile starts earliest
XSPLIT = [128, 128, 256, 256, 256, 256, 256, 256, 256]
F32 = mybir.dt.float32
BF16 = mybir.dt.bfloat16
BF16_NP = ml_dtypes.bfloat16

# store grouping: 2-tile batches, last two tiles stored singly to trim the
# final-store tail
STORE_GROUPS = [2] * 7 + [1, 1]

_PROGRAM_CACHE = {}


def build_program(with_bias: bool) -> bacc.Bacc:
    nc = bacc.Bacc("TRN2", target_bir_lowering=False, debug=False)
    xt = nc.dram_tensor("xt", [D, T], BF16, kind="ExternalInput").ap()
    w = nc.dram_tensor("w", [D, D], BF16, kind="ExternalInput").ap()
    if with_bias:
        beff = nc.dram_tensor("beff", [1, D], BF16, kind="ExternalInput").ap()
    out = nc.dram_tensor("out", [T, D], BF16, kind="ExternalOutput").ap()

    with tile.TileContext(nc) as tc:
        with (
            tc.tile_pool(name="consts", bufs=1) as consts,
            tc.tile_pool(name="wpool", bufs=1) as wpool,
            tc.tile_pool(name="opool", bufs=4) as opool,
            tc.tile_pool(name="po", bufs=POB, space="PSUM") as po,
            tc.tile_pool(name="pwarm", bufs=1, space="PSUM") as pwarm,
        ):
            # PE clock warm-up: ~2.5 us of dummy matmuls on a memset tile so
            # the PE p-state ramps to full speed while the first DMAs land.
            warm_sb = consts.tile([P, WARM_F], BF16)
            nc.vector.memset(warm_sb, 1.0)
            ps_warm = pwarm.tile([P, WARM_F], F32, tag="warm")
            last_warm = None
            for i in range(NWARM):
                last_warm = nc.tensor.matmul(
                    ps_warm,
                    lhsT=warm_sb[:, :P],
                    rhs=warm_sb,
                    start=True,
                    stop=True,
                    skip_group_check=True,
                )

            # Loads on the Sync ring (FIFO): W k0 slice first (128 KB) so the
            # first token tile's first matmul can start earliest, then the
            # first x chunk, then the rest of W, then the remaining x.
            w_sb = wpool.tile([P, KT, D], BF16)
            w_r = w.rearrange("(k p) j -> p k j", p=P)
            nc.sync.dma_start(out=w_sb[:, 0:1, :], in_=w_r[:, 0:1, :])

            xtr = wpool.tile([P, KT, T], BF16)
            xt_r = xt.rearrange("(k p) t -> p k t", p=P)
            xo = XSPLIT[0]
            nc.sync.dma_start(out=xtr[:, :, 0:xo], in_=xt_r[:, :, 0:xo])
            nc.sync.dma_start(out=w_sb[:, 1:KT, :], in_=w_r[:, 1:KT, :])
            for cw in XSPLIT[1:]:
                nc.sync.dma_start(
                    out=xtr[:, :, xo:xo + cw],
                    in_=xt_r[:, :, xo:xo + cw],
                )
                xo += cw

            if with_bias:
                # beff = bv @ Wo + bo folded on host; broadcast-add it into
                # PSUM via a rank-1 matmul with a ones column.
                ones = consts.tile([1, P], BF16)
                nc.vector.memset(ones, 1.0)
                beff_sb = consts.tile([1, D], BF16)
                nc.sync.dma_start(out=beff_sb, in_=beff)

            # Main loop: 4 accumulating bf16 matmuls per 128-token tile,
            # PSUM->SBUF bf16 cast-copies alternating DVE/ACT, stores on the
            # Scalar ring so they overlap the x loads on the Sync ring.
            first_mm = None
            t = 0
            for g, gsz in enumerate(STORE_GROUPS):
                obuf = opool.tile([P, gsz, D], BF16, tag="ob", name=f"ob{g}")
                for s in range(gsz):
                    pso = po.tile([P, D], F32, tag="pso", name=f"pso{t}")
                    for k in range(KT):
                        mm = nc.tensor.matmul(
                            pso,
                            lhsT=xtr[:, k, t * P:(t + 1) * P],
                            rhs=w_sb[:, k, :],
                            start=(k == 0),
                            stop=(k == KT - 1 and not with_bias),
                        )
                        if first_mm is None:
                            first_mm = mm
                            add_dep_helper(
                                first_mm.ins, last_warm.ins,
                                reason="real matmuls after PE warm-up",
                            )
                    if with_bias:
                        nc.tensor.matmul(
                            pso, lhsT=ones, rhs=beff_sb, start=False, stop=True
                        )
                    if t == TT - 1:
                        # last tile: split the cast-copy across DVE and ACT so
                        # the tail drains ~2x faster
                        half = D // 2
                        nc.vector.tensor_copy(
                            out=obuf[:, s, 0:half], in_=pso[:, 0:half]
                        )
                        nc.scalar.copy(
                            out=obuf[:, s, half:D], in_=pso[:, half:D]
                        )
                    elif t % 2 == 0:
                        nc.vector.tensor_copy(out=obuf[:, s, :], in_=pso)
                    else:
                        nc.scalar.copy(out=obuf[:, s, :], in_=pso)
                    t += 1
                base = (t - gsz) * P
                # last two groups store via the (by then idle) Sync ring so
                # the ACT engine isn't serializing copy15 behind store issues
                eng = nc.sync if g >= len(STORE_GROUPS) - 2 else nc.scalar
                eng.dma_start(
                    out=out[base:base + gsz * P, :].rearrange(
                        "(s p) d -> p s d", p=P
                    ),
                    in_=obuf,
                )
    nc.compile()  # bacc: legalizes waits (<=1 per inst via event semaphores)
    return nc


def _get_program(with_bias: bool) -> bacc.Bacc:
    if with_bias not in _PROGRAM_CACHE:
        _PROGRAM_CACHE[with_bias] = build_program(with_bias)
    return _PROGRAM_CACHE[with_bias]


def make_in_maps(x, Wv, bv, Wo, bo):
    x2 = np.asarray(x, dtype=np.float32).reshape(N, D)
    wv_np = np.asarray(Wv, dtype=np.float32)
    wo_np = np.asarray(Wo, dtype=np.float32)
    w_np = np.ascontiguousarray(wv_np @ wo_np).astype(BF16_NP)
    bv_np = np.asarray(bv, dtype=np.float32).reshape(D)
    bo_np = np.asarray(bo, dtype=np.float32).reshape(D)
    with_bias = bool(np.any(bv_np) or np.any(bo_np))
    in_maps = []
    for c in range(N_CORES):
        xt_c = np.ascontiguousarray(x2[c * T:(c + 1) * T].T).astype(BF16_NP)
        m = {"xt": xt_c, "w": w_np}
        if with_bias:
            m["beff"] = (bv_np @ wo_np + bo_np).reshape(1, D).astype(BF16_NP)
        in_maps.append(m)
    return in_maps, with_bias


def kernel(x, H, W, Wq, bq, Wk, bk, Wv, bv, Wo, bo, Woff1, boff1, Woff2, boff2,
           **_ignored):
    in_maps, with_bias = make_in_maps(x, Wv, bv, Wo, bo)
    nc = _get_program(with_bias)
    res = run_bass_kernel_spmd(nc, in_maps, core_ids=list(range(N_CORES)))
    full = np.concatenate(
        [np.asarray(res.results[c]["out"]) for c in range(N_CORES)], axis=0
    )
    return full.reshape(1, N, D).astype(np.float32)


# revision 6
# speedup vs baseline: 1.1235x; 1.1235x over previous
"""Deformable self-attention kernel for Trainium2 (8 NeuronCores).

Structural reduction: the sampling offsets are ``tanh(...) * (2/128)`` with
``|tanh| < 1``, added to *integer* grid coordinates and then rounded.  Since
the perturbation magnitude is < 0.5, ``round(c + d) == c`` always, so the
gather indices are exactly ``arange(N)`` (identity), independent of the data.
Each token attends only to itself at all 7 points; the 7 scores are equal, so
softmax is uniform and the attention output equals ``v``.  The whole module
therefore computes

    out = (x @ Wv + bv) @ Wo + bo = x @ (Wv @ Wo) + (bv @ Wo + bo)

Device strategy (per sharding_hint, row-parallel over the N axis):
  - the weight fold W = Wv @ Wo and the bias fold are computed on host in
    fp32 (134 MFLOP total) and shipped once per core as bf16 [D, D];
  - each core gets 2048 tokens of x, fed pre-transposed ([D, T] layout) and
    pre-cast to bf16 - layout/dtype marshaling done while sharding;
  - the main [2048, 512] @ [512, 512] matmul runs in bf16 (1 cycle/row on
    the PE, fp32 PSUM accumulate), which is the PE roofline for this GEMM;
  - outputs are stored as bf16 and upcast on host; total HBM traffic per
    core is 2 MB (x) + 0.5 MB (W) + 2 MB (out) = 4.5 MB vs 10 MB for the
    fp32 version;
  - the PE clock needs ~3 us of sustained work to ramp to full speed, so a
    chain of dummy matmuls warms it up while the first DMAs land;
  - loads stream on the Sync hwdge ring (small W k0-slice first so the
    first token tile can start after ~0.6 MB, not after all 2.5 MB),
    stores stream on the Scalar hwdge ring so they overlap the loads;
  - PSUM->SBUF copies (with the fp32->bf16 cast) alternate DVE/ACT.
"""

import os
import sys

import numpy as np

for _p in ("/opt/trn_rl_repo", "/root/.axon_site/_ro/trn_rl_repo"):
    if os.path.isdir(_p) and _p not in sys.path:
        sys.path.append(_p)

import ml_dtypes
import concourse.bass as bass
import concourse.mybir as mybir
import concourse.tile as tile
from concourse import bacc
from concourse.bass_utils import run_bass_kernel_spmd
from concourse.tile import add_dep_helper

N_CORES = 8
N = 16384          # tokens (128 x 128 grid)
D = 512            # d_model
T = N // N_CORES   # tokens per core
P = 128            # partitions
TT = T // P        # token tiles per core (16)
KT = D // P        # contraction tiles (4)
POB = 4            # main psum bufs
NWARM = 17         # PE clock warm-up matmuls (bridge until first x lands)
WARM_F = 256       # warm-up moving size
# input DMA split; 256-col chunks keep descriptors at 512 B (the threshold
# below which the DMA engines pay a 2x latency penalty)
XSPLIT = [256] * 8
F32 = mybir.dt.float32
BF16 = mybir.dt.bfloat16
BF16_NP = ml_dtypes.bfloat16

# store grouping: 2-tile batches, last two tiles stored singly to trim the
# final-store tail
STORE_GROUPS = [2] * 7 + [1, 1]

_PROGRAM_CACHE = {}


def build_program(with_bias: bool) -> bacc.Bacc:
    nc = bacc.Bacc("TRN2", target_bir_lowering=False, debug=False)
    xt = nc.dram_tensor("xt", [D, T], BF16, kind="ExternalInput").ap()
    w = nc.dram_tensor("w", [D, D], BF16, kind="ExternalInput").ap()
    if with_bias:
        beff = nc.dram_tensor("beff", [1, D], BF16, kind="ExternalInput").ap()
    out = nc.dram_tensor("out", [T, D], BF16, kind="ExternalOutput").ap()

    with tile.TileContext(nc) as tc:
        with (
            tc.tile_pool(name="consts", bufs=1) as consts,
            tc.tile_pool(name="wpool", bufs=1) as wpool,
            tc.tile_pool(name="opool", bufs=4) as opool,
            tc.tile_pool(name="po", bufs=POB, space="PSUM") as po,
            tc.tile_pool(name="pwarm", bufs=1, space="PSUM") as pwarm,
        ):
            # PE clock warm-up: ~2.5 us of dummy matmuls on a memset tile so
            # the PE p-state ramps to full speed while the first DMAs land.
            warm_sb = consts.tile([P, WARM_F], BF16)
            nc.vector.memset(warm_sb, 1.0)
            ps_warm = pwarm.tile([P, WARM_F], F32, tag="warm")
            last_warm = None
            for i in range(NWARM):
                last_warm = nc.tensor.matmul(
                    ps_warm,
                    lhsT=warm_sb[:, :P],
                    rhs=warm_sb,
                    start=True,
                    stop=True,
                    skip_group_check=True,
                )

            # Loads on the Sync ring (FIFO): W k0 slice first (128 KB) so the
            # first token tile's first matmul can start earliest, then the
            # first x chunk, then the rest of W, then the remaining x.
            w_sb = wpool.tile([P, KT, D], BF16)
            w_r = w.rearrange("(k p) j -> p k j", p=P)
            nc.sync.dma_start(out=w_sb[:, 0:1, :], in_=w_r[:, 0:1, :])

            xtr = wpool.tile([P, KT, T], BF16)
            xt_r = xt.rearrange("(k p) t -> p k t", p=P)
            xo = XSPLIT[0]
            nc.sync.dma_start(out=xtr[:, :, 0:xo], in_=xt_r[:, :, 0:xo])
            nc.sync.dma_start(out=w_sb[:, 1:KT, :], in_=w_r[:, 1:KT, :])
            for cw in XSPLIT[1:]:
                nc.sync.dma_start(
                    out=xtr[:, :, xo:xo + cw],
                    in_=xt_r[:, :, xo:xo + cw],
                )
                xo += cw

            if with_bias:
                # beff = bv @ Wo + bo folded on host; broadcast-add it into
                # PSUM via a rank-1 matmul with a ones column.
                ones = consts.tile([1, P], BF16)
                nc.vector.memset(ones, 1.0)
                beff_sb = consts.tile([1, D], BF16)
                nc.sync.dma_start(out=beff_sb, in_=beff)

            # Main loop: 4 accumulating bf16 matmuls per 128-token tile,
            # PSUM->SBUF bf16 cast-copies alternating DVE/ACT, stores on the
            # Scalar ring so they overlap the x loads on the Sync ring.
            first_mm = None
            t = 0
            for g, gsz in enumerate(STORE_GROUPS):
                obuf = opool.tile([P, gsz, D], BF16, tag="ob", name=f"ob{g}")
                for s in range(gsz):
                    pso = po.tile([P, D], F32, tag="pso", name=f"pso{t}")
                    for k in range(KT):
                        mm = nc.tensor.matmul(
                            pso,
                            lhsT=xtr[:, k, t * P:(t + 1) * P],
                            rhs=w_sb[:, k, :],
                            start=(k == 0),
                            stop=(k == KT - 1 and not with_bias),
                        )
                        if first_mm is None:
                            first_mm = mm
                            add_dep_helper(
                                first_mm.ins, last_warm.ins,
                                reason="real matmuls after PE warm-up",
                            )
                    if with_bias:
                        nc.tensor.matmul(
                            pso, lhsT=ones, rhs=beff_sb, start=False, stop=True
                        )
                    if t == TT - 1:
                        # last tile: split the cast-copy across DVE and ACT so
                        # the tail drains ~2x faster
                        half = D // 2
                        nc.vector.tensor_copy(
                            out=obuf[:, s, 0:half], in_=pso[:, 0:half]
                        )
                        nc.scalar.copy(
                            out=obuf[:, s, half:D], in_=pso[:, half:D]
                        )
                    elif t % 2 == 0:
                        nc.vector.tensor_copy(out=obuf[:, s, :], in_=pso)
                    else:
                        nc.scalar.copy(out=obuf[:, s, :], in_=pso)
                    t += 1
                base = (t - gsz) * P
                # last two groups store via the (by then idle) Sync ring so
                # the ACT engine isn't serializing copy15 behind store issues
                eng = nc.sync if g >= len(STORE_GROUPS) - 2 else nc.scalar
                eng.dma_start(
                    out=out[base:base + gsz * P, :].rearrange(
                        "(s p) d -> p s d", p=P
                    ),
                    in_=obuf,
                )
    nc.compile()  # bacc: legalizes waits (<=1 per inst via event semaphores)
    return nc


def _get_program(with_bias: bool) -> bacc.Bacc:
    if with_bias not in _PROGRAM_CACHE:
        _PROGRAM_CACHE[with_bias] = build_program(with_bias)
    return _PROGRAM_CACHE[with_bias]


def make_in_maps(x, Wv, bv, Wo, bo):
    x2 = np.asarray(x, dtype=np.float32).reshape(N, D)
    wv_np = np.asarray(Wv, dtype=np.float32)
    wo_np = np.asarray(Wo, dtype=np.float32)
    w_np = np.ascontiguousarray(wv_np @ wo_np).astype(BF16_NP)
    bv_np = np.asarray(bv, dtype=np.float32).reshape(D)
    bo_np = np.asarray(bo, dtype=np.float32).reshape(D)
    with_bias = bool(np.any(bv_np) or np.any(bo_np))
    in_maps = []
    for c in range(N_CORES):
        xt_c = np.ascontiguousarray(x2[c * T:(c + 1) * T].T).astype(BF16_NP)
        m = {"xt": xt_c, "w": w_np}
        if with_bias:
            m["beff"] = (bv_np @ wo_np + bo_np).reshape(1, D).astype(BF16_NP)
        in_maps.append(m)
    return in_maps, with_bias


def kernel(x, H, W, Wq, bq, Wk, bk, Wv, bv, Wo, bo, Woff1, boff1, Woff2, boff2,
           **_ignored):
    in_maps, with_bias = make_in_maps(x, Wv, bv, Wo, bo)
    nc = _get_program(with_bias)
    res = run_bass_kernel_spmd(nc, in_maps, core_ids=list(range(N_CORES)))
    full = np.concatenate(
        [np.asarray(res.results[c]["out"]) for c in range(N_CORES)], axis=0
    )
    return full.reshape(1, N, D).astype(np.float32)
